# revision 17
# baseline (speedup 1.0000x reference)
"""NeighborSample Trainium2 kernel: all stores port-complete seg-32.

Key facts (HW-probed):
  - SBUF AXI port = ((p>>2)&7)<<1 | (p>>6): partitions 0-63 reach only the
    8 even ports (~204 GB/s); a stride-4 walk over 32 partitions covers all
    16 ports (~430 GB/s single queue).
  - Engine fan-out = largest divisor <= 16 of the OUTERMOST AP dim count.

Layout: row r (0..63, full padded width 68*192) -> partition
4*(r%32) + r//32, so a 32-row block walk (stride 4, count 32) covers all
16 ports. The output is written PADDED: out_p[h', i, j, w, c] with
h' = h + 2, shape [68,...]; rows 0,1,66,67 receive garbage from full
32-row stores whose h range spills past the real [0,64) — the host
slices them off. This keeps every main store at outer count 32 (16
engines, 16 ports); there are no odd-sized segment chops.

Queues: sync handles row block 0, scalar block 1; both queues issue
byte-identical DMA sequences (load, 3 zero blocks, 5 seg-32 stores) so
the per-engine descriptor streams stay in lockstep — any per-engine
count asymmetry (measured) desyncs the engine->port pairing and
stretches every packet ~20%. Load is issued before the memset wait; the
tiny zero-block stores drain inside the load's HBM-read latency window.

Host unshard: slice h' [2:66], transpose [h,i,j,w,c] -> [h,w,i,j,c].
"""

import sys

for _p in ("/opt/trn_rl_repo",):
    if _p not in sys.path:
        sys.path.insert(0, _p)

import numpy as np

import concourse.bass as bass
import concourse.mybir as mybir
from concourse.bass_utils import run_bass_kernel_spmd

B = 8
H = W = 64
C = 192
K = 5
PAD = 2
HP = H + 2 * PAD         # 68 padded output rows
ROW = (W + 2 * PAD) * C  # 13056 elems per partition (full padded row)
RUN = W * C              # 12288 elems: one (w,c) descriptor
PADE = PAD * C           # 384 pad elems at each end of a row
OH = K * K * W * C       # 307200 (h' stride)
OI = K * W * C           # 61440  (i stride)
OJ = W * C               # 12288  (j stride)


PP = ROW                 # partition pitch in AP flat element space
HB = H // 2              # 32 rows per partition block

N_PER_Q = 5              # stores (per queue), each inc 16
DMA_TOTAL = 16 * 2 * N_PER_Q


def build_nc() -> bass.Bass:
    nc = bass.Bass()
    x = nc.declare_dram_parameter("x", [H, W, C], mybir.dt.float32, isOutput=False)
    out = nc.declare_dram_parameter(
        "out", [HP, K, K, W, C], mybir.dt.float32, isOutput=True
    )

    with (
        nc.Block() as block,
        nc.semaphore("dve_sem") as dve_sem,
        nc.semaphore("dma_sem") as dma_sem,
        nc.semaphore("ld0_sem") as ld0_sem,
        nc.semaphore("ld1_sem") as ld1_sem,
        nc.sbuf_tensor("buf", [128, ROW], mybir.dt.float32) as buf,
    ):

        @block.vector
        def _(vector):
            vector.memset(bass.AP(buf, 0, [[PP, 128], [1, PADE]]), 0.0).then_inc(
                dve_sem, 1
            )
            vector.memset(
                bass.AP(buf, ROW - PADE, [[PP, 128], [1, PADE]]), 0.0
            ).then_inc(dve_sem, 1)

        def emit(eng, blk, ld_sem):
            # load first (independent of the memsets): x rows
            # [32*blk, 32*blk+32) -> partitions blk+4m
            eng.dma_start(
                out=bass.AP(buf, blk * PP + PADE, [[4 * PP, HB], [1, RUN]]),
                in_=bass.AP(x, blk * HB * RUN, [[RUN, HB], [1, RUN]]),
            ).then_inc(ld_sem, 16)
            # row-OOB zero blocks are NOT written on device — the host
            # unshard step zeroes those slices (they are fixed positions)
            eng.wait_ge(dve_sem, 2)  # pad-column memsets before stores issue
            # no load wait: load and stores are outer-32 walks over the same
            # partitions, so each engine's store descriptors queue behind its
            # own load descriptors in the same FIFO ring
            # 5 full-block stores: h' = r - i + 4, r in [32*blk, 32*blk+32)
            for i in range(K):
                h0 = HB * blk - i + 2 * PAD
                eng.dma_start(
                    out=bass.AP(
                        out, h0 * OH + i * OI, [[OH, HB], [OJ, K], [1, RUN]]
                    ),
                    in_=bass.AP(
                        buf, blk * PP, [[4 * PP, HB], [C, K], [1, RUN]]
                    ),
                ).then_inc(dma_sem, 16)
            eng.wait_ge(dma_sem, DMA_TOTAL)
            eng.wait_ge(ld0_sem, 16)
            eng.wait_ge(ld1_sem, 16)

        @block.sync
        def _(sync):
            emit(sync, 0, ld0_sem)

        @block.scalar
        def _(scalar):
            emit(scalar, 1, ld1_sem)

    return nc


_NC_CACHE = None


def kernel(x) -> np.ndarray:
    global _NC_CACHE
    x = np.asarray(x, dtype=np.float32)
    assert x.shape == (B, H, W, C), x.shape
    if _NC_CACHE is None:
        _NC_CACHE = build_nc()
    in_maps = [{"x": np.ascontiguousarray(x[i])} for i in range(B)]
    res = run_bass_kernel_spmd(_NC_CACHE, in_maps, list(range(B)))
    outs = [
        res.results[i]["out"]
        .reshape(HP, K, K, W, C)[PAD : PAD + H]
        .transpose(0, 3, 1, 2, 4)
        .reshape(H * W, K, K, C)
        for i in range(B)
    ]
    full = np.concatenate(outs, axis=0)
    # row-OOB zeros (source row h+i-2 outside [0,64)) — fixed positions the
    # device skips; their DRAM is uninitialized until this fill
    v = full.reshape(B, H, W, K, K, C)
    v[:, :PAD, :, 0] = 0.0       # i=0: h 0,1
    v[:, :1, :, 1] = 0.0         # i=1: h 0
    v[:, H - 1 :, :, 3] = 0.0    # i=3: h 63
    v[:, H - PAD :, :, 4] = 0.0  # i=4: h 62,63
    return full


# revision 18
# speedup vs baseline: 1.1818x; 1.1818x over previous
"""NeighborSample Trainium2 kernel: all stores port-complete seg-32.

Key facts (HW-probed):
  - SBUF AXI port = ((p>>2)&7)<<1 | (p>>6): partitions 0-63 reach only the
    8 even ports (~204 GB/s); a stride-4 walk over 32 partitions covers all
    16 ports (~430 GB/s single queue).
  - Engine fan-out = largest divisor <= 16 of the OUTERMOST AP dim count.

Layout: row r (0..63, full padded width 68*192) -> partition
4*(r%32) + r//32, so a 32-row block walk (stride 4, count 32) covers all
16 ports. The output is written PADDED: out_p[h', i, j, w, c] with
h' = h + 2, shape [68,...]; rows 0,1,66,67 receive garbage from full
32-row stores whose h range spills past the real [0,64) — the host
slices them off. This keeps every main store at outer count 32 (16
engines, 16 ports); there are no odd-sized segment chops.

Queues: sync handles row block 0, scalar block 1; both queues issue
byte-identical DMA sequences (load, 3 zero blocks, 5 seg-32 stores) so
the per-engine descriptor streams stay in lockstep — any per-engine
count asymmetry (measured) desyncs the engine->port pairing and
stretches every packet ~20%. Load is issued before the memset wait; the
tiny zero-block stores drain inside the load's HBM-read latency window.

Host unshard: slice h' [2:66], transpose [h,i,j,w,c] -> [h,w,i,j,c].
"""

import sys

for _p in ("/opt/trn_rl_repo",):
    if _p not in sys.path:
        sys.path.insert(0, _p)

import numpy as np

import concourse.bass as bass
import concourse.mybir as mybir
from concourse.bass_utils import run_bass_kernel_spmd

B = 8
H = W = 64
C = 192
K = 5
PAD = 2
HP = H + 2 * PAD         # 68 padded output rows
ROW = (W + 2 * PAD) * C  # 13056 elems per partition (full padded row)
RUN = W * C              # 12288 elems: one (w,c) descriptor
PADE = PAD * C           # 384 pad elems at each end of a row
OH = K * K * W * C       # 307200 (h' stride)
OI = K * W * C           # 61440  (i stride)
OJ = W * C               # 12288  (j stride)
ZN = 480                 # elems per partition per zero row (128*480 = one OI block)
ZB = 2 * ZN              # zbuf free size: two zero-row slots

PP = ROW                 # partition pitch in AP flat element space
HB = H // 2              # 32 rows per partition block

N_PER_Q = 2 + 5          # zeros + stores (per queue), each inc 16
DMA_TOTAL = 16 * 2 * N_PER_Q


def build_nc() -> bass.Bass:
    nc = bass.Bass()
    x = nc.declare_dram_parameter("x", [H, W, C], mybir.dt.float32, isOutput=False)
    out = nc.declare_dram_parameter(
        "out", [HP, K, K, W, C], mybir.dt.float32, isOutput=True
    )

    # (i, h'_start, n_rows) zero blocks; consecutive-h' pairs merged
    zsync = [(0, 2, 2), (1, 2, 1)]        # top edge
    zscal = [(4, 64, 2), (3, 65, 1)]      # bottom edge

    with (
        nc.Block() as block,
        nc.semaphore("dve_sem") as dve_sem,
        nc.semaphore("dma_sem") as dma_sem,
        nc.semaphore("ld0_sem") as ld0_sem,
        nc.semaphore("ld1_sem") as ld1_sem,
        nc.sbuf_tensor("buf", [128, ROW], mybir.dt.float32) as buf,
        nc.sbuf_tensor("zbuf", [128, ZB], mybir.dt.float32) as zbuf,
    ):

        @block.vector
        def _(vector):
            vector.memset(bass.AP(buf, 0, [[PP, 128], [1, PADE]]), 0.0).then_inc(
                dve_sem, 1
            )
            vector.memset(
                bass.AP(buf, ROW - PADE, [[PP, 128], [1, PADE]]), 0.0
            ).then_inc(dve_sem, 1)
            vector.memset(bass.AP(zbuf, 0, [[ZB, 128], [1, ZB]]), 0.0).then_inc(
                dve_sem, 1
            )

        def emit(eng, blk, zeros, ld_sem):
            # load first (independent of the memsets): x rows
            # [32*blk, 32*blk+32) -> partitions blk+4m
            eng.dma_start(
                out=bass.AP(buf, blk * PP + PADE, [[4 * PP, HB], [1, RUN]]),
                in_=bass.AP(x, blk * HB * RUN, [[RUN, HB], [1, RUN]]),
            ).then_inc(ld_sem, 16)
            # zero blocks drain during the load's HBM-read latency window
            eng.wait_ge(dve_sem, 3)
            for i, hz, nr in zeros:
                eng.dma_start(
                    out=bass.AP(
                        out, hz * OH + i * OI, [[ZN, 128], [OH, nr], [1, ZN]]
                    ),
                    in_=bass.AP(zbuf, 0, [[ZB, 128], [ZN, nr], [1, ZN]]),
                ).then_inc(dma_sem, 16)
            # no load wait: load and stores are outer-32 walks over the same
            # partitions, so each engine's store descriptors queue behind its
            # own load descriptors in the same FIFO ring
            # 5 full-block stores: h' = r - i + 4, r in [32*blk, 32*blk+32)
            for i in range(K):
                h0 = HB * blk - i + 2 * PAD
                eng.dma_start(
                    out=bass.AP(
                        out, h0 * OH + i * OI, [[OH, HB], [OJ, K], [1, RUN]]
                    ),
                    in_=bass.AP(
                        buf, blk * PP, [[4 * PP, HB], [C, K], [1, RUN]]
                    ),
                ).then_inc(dma_sem, 16)
            eng.wait_ge(dma_sem, DMA_TOTAL)
            eng.wait_ge(ld0_sem, 16)
            eng.wait_ge(ld1_sem, 16)

        @block.sync
        def _(sync):
            emit(sync, 0, zsync, ld0_sem)

        @block.scalar
        def _(scalar):
            emit(scalar, 1, zscal, ld1_sem)

    return nc


_NC_CACHE = None


def kernel(x) -> np.ndarray:
    global _NC_CACHE
    x = np.asarray(x, dtype=np.float32)
    assert x.shape == (B, H, W, C), x.shape
    if _NC_CACHE is None:
        _NC_CACHE = build_nc()
    in_maps = [{"x": np.ascontiguousarray(x[i])} for i in range(B)]
    res = run_bass_kernel_spmd(_NC_CACHE, in_maps, list(range(B)))
    outs = [
        res.results[i]["out"]
        .reshape(HP, K, K, W, C)[PAD : PAD + H]
        .transpose(0, 3, 1, 2, 4)
        .reshape(H * W, K, K, C)
        for i in range(B)
    ]
    return np.concatenate(outs, axis=0)
